# revision 25
# baseline (speedup 1.0000x reference)
"""DistillKL ('w' branch) fused Trainium2 kernel.

Math (per batch row b, C=1000 classes, T=4):
  x = y / T
  t3[i]   = (1/C) * sum_k w[k] * relu(x[i] - x[k])        # [C,C] intermediate, fused on-chip
  choice  = w * exp(-t3) + (1-w)
  ex      = exp(x)
  nsum    = sum_k (1-w[k]) * ex[k]
  p       = ex / (choice * nsum + ex)                      # p in (0, 1]
  (computed for y_s -> p_s and y_t -> p_t)
  pos     = |0.5*p_t - p_s + 0.5| ** 0.25
  neg     = |0.5*p_t - p_s|
  expt    = w ? pos : neg
  row[b]  = sum_i p_t * max(expt * ln(p_s), ln(1e-7))
  loss    = -mean_b(row) * T^2

Sharding: data-parallel over the batch dim. 64 rows -> 8 cores x 8 rows.
Each core returns its 8 row sums; the host averages and scales.

On-chip dataflow per core (8 rows, 2 inputs):
  - k padded 1000 -> 8 tiles x 128 (pad rows get w=0, contribute nothing)
  - x_bcast[128, 1000] built by a ones[1,128] matmul broadcast (PE), copied to SBUF
  - relu tile [128k, 1000i] = Relu(x_bcast + bias(-x_k)) on ACT (or DVE tensor_scalar)
  - PE contracts each relu tile with the w column (lhsT [128,1]) accumulating
    into PSUM acc[8, 1000] at partition offset b (two N=500 chains per row)
  - elementwise tail in [8, 1000] row layout, fused reductions via
    tensor_tensor_reduce / scalar_tensor_tensor accum_out
"""

import numpy as np

import concourse.bass as bass
import concourse.bacc as bacc
import concourse.tile as tile
from concourse import mybir
from concourse.bass_utils import run_bass_kernel_spmd
from concourse.masks import make_identity

B, C = 64, 1000
NCORES = 8
BPC = B // NCORES          # rows per core
P = 128                    # partitions
KT = 8                     # k tiles (7 full + 1 of 104)
KLAST = C - (KT - 1) * P   # 104
INV_T = 0.25               # 1/T
LOG_EPS = float(np.log(1e-7))

F32 = mybir.dt.float32
BF16 = mybir.dt.bfloat16
AF = mybir.ActivationFunctionType
ALU = mybir.AluOpType

# Producer engine per (input, batch, ktile) index: True -> DVE, False -> ACT.
# Flat index = (inp * BPC + b) * KT + kt.  Tuned after tracing.
N_PROD = 2 * BPC * KT


# 3-way producer split, tuned from trace rates (ACT ~1.11us, DVE ~0.62us,
# GpSimd ~1.4us per [128,1000] tile)
def _producer_engine(flat_idx: int) -> str:
    m = flat_idx % 16
    if m in (0, 5):
        return "gpsimd"
    if m in (1, 3, 6, 8, 10, 12, 14, 15, 11):
        return "dve"
    return "act"


def _ksz(kt: int) -> int:
    return KLAST if kt == KT - 1 else P


def build_kernel_body(ctx, tc, ys_d, yt_d, w_d, out_d):
    import os

    skip_mm = bool(int(os.environ.get("DISTILL_SKIP_MM", "0")))
    skip_relu = bool(int(os.environ.get("DISTILL_SKIP_RELU", "0")))
    skip_tp = bool(int(os.environ.get("DISTILL_SKIP_TP", "0")))
    nc = tc.nc
    consts = ctx.enter_context(tc.tile_pool(name="consts", bufs=1))
    rows = ctx.enter_context(tc.tile_pool(name="rows", bufs=1))
    cols = ctx.enter_context(tc.tile_pool(name="cols", bufs=1))
    xb_pool = ctx.enter_context(tc.tile_pool(name="xb", bufs=3))
    relu_pool = ctx.enter_context(tc.tile_pool(name="relu", bufs=8))
    tails = ctx.enter_context(tc.tile_pool(name="tails", bufs=1))
    ps_stage = ctx.enter_context(tc.tile_pool(name="ps_stage", bufs=2, space="PSUM"))
    ps_acc = ctx.enter_context(tc.tile_pool(name="ps_acc", bufs=1, space="PSUM"))

    # ---- constants ----
    ident = consts.tile([P, P], F32)
    make_identity(nc, ident)

    # ---- load rows, scale x = y/4 ----
    ys_raw = rows.tile([BPC, C], F32, tag="ys_raw", name="ys_raw")
    yt_raw = rows.tile([BPC, C], F32, tag="yt_raw", name="yt_raw")
    w_row = rows.tile([BPC, C], F32, tag="w_row", name="w_row")
    nc.sync.dma_start(out=ys_raw, in_=ys_d.ap())
    nc.sync.dma_start(out=yt_raw, in_=yt_d.ap())
    nc.sync.dma_start(out=w_row, in_=w_d.ap())

    x_rows = []
    for name, raw in (("xs", ys_raw), ("xt", yt_raw)):
        xr = rows.tile([BPC, C], F32, tag=name, name=name)
        nc.vector.tensor_scalar_mul(xr, raw, INV_T)
        x_rows.append(xr)

    # x rows bounced to DRAM: a 0-stride partition AP can broadcast a DRAM
    # row across all 128 partitions in one DMA (not legal from SBUF)
    dram = ctx.enter_context(tc.tile_pool(name="dram", bufs=1, space="DRAM"))
    xdr = dram.tile([2, BPC, C], BF16, name="xdr")
    for inp in range(2):
        xbf = rows.tile([BPC, C], BF16, tag=f"xbf{inp}", name=f"xbf{inp}")
        nc.vector.tensor_copy(xbf, x_rows[inp])
        nc.sync.dma_start(out=xdr[inp], in_=xbf)

    omw = rows.tile([BPC, C], F32, tag="omw", name="omw")  # 1 - w
    nc.vector.tensor_scalar(
        omw, w_row, -1.0, 1.0, op0=ALU.mult, op1=ALU.add
    )

    # ---- column layouts via PE transpose: [128k, BPC] per ktile ----
    # negx_cols[inp][kt][:, b] = -x[b, kt*128 + p]
    # wplace[kt][:, b, m] = w[b, kt*128 + p] if m == b else 0   (stationary
    # [128, BPC] slices let the matmul write row b of the shared [BPC, C]
    # accumulator directly; other rows accumulate +0)
    negx_cols = [[], []]
    wplace = []
    for kt in range(KT):
        ksz = _ksz(kt)
        ksl = slice(kt * P, kt * P + ksz)
        for inp in range(2):
            pst = ps_stage.tile([P, BPC], F32, tag="xb_ps", name="pst")
            if skip_tp:
                nc.vector.memset(pst, 0.123)
            else:
                nc.tensor.transpose(pst[:ksz, :], x_rows[inp][:, ksl], ident[:BPC, :BPC])
            col = cols.tile([P, BPC], F32, tag=f"negx{inp}_{kt}", name=f"negx{inp}_{kt}")
            if ksz < P:
                # pad memset must start on a quadrant boundary; valid rows
                # 96:ksz are rewritten by the scalar_mul below
                nc.vector.memset(col[96:, :], 0.0)
            nc.vector.tensor_scalar_mul(col[:ksz, :], pst[:ksz, :], -1.0)
            negx_cols[inp].append(col)
        pst = ps_stage.tile([P, BPC], F32, tag="xb_ps", name="pst")
        if skip_tp:
            nc.vector.memset(pst, 0.5)
        else:
            nc.tensor.transpose(pst[:ksz, :], w_row[:, ksl], ident[:BPC, :BPC])
        wp = cols.tile([P, BPC, BPC], BF16, tag=f"wp_{kt}", name=f"wp_{kt}")
        nc.gpsimd.memset(wp, 0.0)
        # all 8 diagonal slots in one strided copy: flat free offset 9*b
        wp_flat = wp.rearrange("p a b -> p (a b)")
        diag = bass.AP(
            tensor=wp_flat.tensor,
            offset=wp_flat.offset,
            ap=[list(wp_flat.ap[0][:2]), [(BPC + 1) * wp_flat.ap[1][0], BPC]],
        )
        nc.vector.tensor_copy(diag[:ksz, :], pst[:ksz, :])
        wplace.append(wp)

    # ---- accumulators: acc[inp][b, i] = sum_k w[b,k] relu(x[b,i]-x[b,k]) ----
    accs = [
        ps_acc.tile([BPC, C], F32, tag="acc_s", name="acc_s"),
        ps_acc.tile([BPC, C], F32, tag="acc_t", name="acc_t"),
    ]

    if skip_mm:
        zacc = rows.tile([BPC, C], F32, tag="zacc", name="zacc")
        nc.vector.memset(zacc, 1.0)
        accs = [zacc, zacc]

    halves = ((0, 512), (512, 1000))  # psum bank = 512 f32; matmul can't cross
    for inp in range(2):
        for b in range(BPC):
            # broadcast x row b across all 128 partitions (0-stride DMA read)
            xb = xb_pool.tile([P, C], BF16, tag="xb", name="xb")
            row = xdr[inp, b : b + 1, :]
            bc = bass.AP(
                tensor=row.tensor,
                offset=row.offset,
                ap=[[0, P]] + list(row.ap[1:]),
            )
            nc.sync.dma_start(out=xb, in_=bc)

            for kt in range(KT):
                rd = relu_pool.tile([P, C], BF16, tag="rd", name="rd")
                bias_col = negx_cols[inp][kt][:, b : b + 1]
                eng = _producer_engine((inp * BPC + b) * KT + kt)
                if skip_relu:
                    nc.vector.memset(rd, 0.25)
                elif eng == "dve":
                    nc.vector.tensor_scalar(
                        rd, xb, bias_col, 0.0, op0=ALU.add, op1=ALU.max
                    )
                elif eng == "gpsimd":
                    nc.gpsimd.tensor_scalar(
                        rd, xb, bias_col, 0.0, op0=ALU.add, op1=ALU.max
                    )
                else:
                    nc.scalar.activation(rd, xb, AF.Relu, bias=bias_col, scale=1.0)
                if not skip_mm:
                    for h0, h1 in halves:
                        nc.tensor.matmul(
                            accs[inp][:, h0:h1],
                            wplace[kt][:, b, :],
                            rd[:, h0:h1],
                            start=(b == 0 and kt == 0),
                            stop=(b == BPC - 1 and kt == KT - 1),
                        )

    # ---- elementwise tail in [BPC, C] layout ----
    def t(tag):
        return tails.tile([BPC, C], F32, tag=tag, name=tag)

    ps = []  # p_s, p_t
    exs = []  # exp(x_s), exp(x_t)
    for inp, nm in ((0, "s"), (1, "t")):
        ex = t(f"ex_{nm}")
        nc.scalar.activation(ex, x_rows[inp], AF.Exp)
        exs.append(ex)
        # nsum[b] = sum_k (1-w) * ex   (tensor_tensor_reduce wedges HW; use 2 ops)
        junk = t(f"junk_{nm}")
        nsum = tails.tile([BPC, 1], F32, tag=f"nsum_{nm}", name=f"nsum_{nm}")
        nc.vector.tensor_mul(junk, ex, omw)
        nc.vector.reduce_sum(nsum, junk, axis=mybir.AxisListType.X)
        # choice = w * exp(-t3) + (1-w);  t3 = acc / C
        e3 = t(f"e3_{nm}")
        nc.scalar.activation(e3, accs[inp], AF.Exp, scale=-1.0 / C)
        ch = t(f"ch_{nm}")
        nc.gpsimd.tensor_mul(ch, e3, w_row)
        nc.gpsimd.tensor_add(ch, ch, omw)
        # den = choice * nsum + ex ; p = ex / den
        den = t(f"den_{nm}")
        nc.vector.scalar_tensor_tensor(
            out=den, in0=ch, scalar=nsum, in1=ex, op0=ALU.mult, op1=ALU.add
        )
        rden = t(f"rden_{nm}")
        rscr = t("rscr")
        nc.vector.reciprocal_approx_accurate(rden, den, rscr)
        p = t(f"p_{nm}")
        nc.vector.tensor_mul(p, ex, rden)
        ps.append(p)

    p_s, p_t = ps
    # d1 = 0.5 * p_t - p_s
    d1 = t("d1")
    nc.vector.scalar_tensor_tensor(
        out=d1, in0=p_t, scalar=0.5, in1=p_s, op0=ALU.mult, op1=ALU.subtract
    )
    negv = t("negv")  # |d1| = max(-d1, d1)
    nc.vector.scalar_tensor_tensor(
        out=negv, in0=d1, scalar=-1.0, in1=d1, op0=ALU.mult, op1=ALU.max
    )
    posv = t("posv")  # |d1 + 0.5| ** 0.25
    half_bias = tails.tile([BPC, 1], F32, tag="half_bias", name="half_bias")
    nc.vector.memset(half_bias, 0.5)
    nc.scalar.activation(posv, d1, AF.Abs, bias=half_bias)
    nc.scalar.activation(posv, posv, AF.Sqrt)
    nc.scalar.activation(posv, posv, AF.Sqrt)
    # expt = w ? posv : negv  (w in {0,1}: negv + w*(posv-negv))
    expt = t("expt")
    nc.vector.tensor_sub(expt, posv, negv)
    nc.vector.tensor_mul(expt, expt, w_row)
    nc.vector.tensor_add(expt, expt, negv)
    # l = max(expt * ln(p_s), ln(1e-7)); row[b] = sum_i p_t * l
    lp = t("lp")
    nc.scalar.activation(lp, p_s, AF.Ln)
    l = t("l")
    nc.vector.tensor_mul(l, expt, lp)
    nc.vector.tensor_scalar_max(l, l, LOG_EPS)
    junk2 = t("junk2")
    rowsum = tails.tile([BPC, 1], F32, tag="rowsum", name="rowsum")
    nc.vector.tensor_mul(junk2, l, p_t)
    nc.vector.reduce_sum(rowsum, junk2, axis=mybir.AxisListType.X)
    nc.sync.dma_start(out=out_d.ap(), in_=rowsum)


_CACHE = {}


def _get_nc():
    if "nc" not in _CACHE:
        nc = bacc.Bacc(
            "TRN2", target_bir_lowering=False, debug=False, num_devices=NCORES
        )
        ys_d = nc.dram_tensor("y_s", [BPC, C], F32, kind="ExternalInput")
        yt_d = nc.dram_tensor("y_t", [BPC, C], F32, kind="ExternalInput")
        w_d = nc.dram_tensor("w", [BPC, C], F32, kind="ExternalInput")
        out_d = nc.dram_tensor("out", [BPC, 1], F32, kind="ExternalOutput")
        from contextlib import ExitStack

        with tile.TileContext(nc) as tc, ExitStack() as ctx:
            build_kernel_body(ctx, tc, ys_d, yt_d, w_d, out_d)
        nc.compile()
        _CACHE["nc"] = nc
    return _CACHE["nc"]


def kernel(y_s, y_t, w, _trace=False):
    nc = _get_nc()
    y_s = np.ascontiguousarray(y_s, dtype=np.float32)
    y_t = np.ascontiguousarray(y_t, dtype=np.float32)
    w = np.ascontiguousarray(w, dtype=np.float32)
    in_maps = [
        {
            "y_s": y_s[c * BPC : (c + 1) * BPC],
            "y_t": y_t[c * BPC : (c + 1) * BPC],
            "w": w[c * BPC : (c + 1) * BPC],
        }
        for c in range(NCORES)
    ]
    trace_kwargs = (
        {"trace": True, "trace_cores": list(range(NCORES))} if _trace else {}
    )
    res = run_bass_kernel_spmd(
        nc, in_maps, core_ids=list(range(NCORES)), **trace_kwargs
    )
    _CACHE["last_result"] = res
    rows = np.concatenate([res.results[c]["out"].reshape(-1) for c in range(NCORES)])
    return np.asarray(-16.0 * rows.mean(), dtype=np.float32)


# revision 26
# speedup vs baseline: 2.6382x; 2.6382x over previous
"""DistillKL ('w' branch) fused Trainium2 kernel.

Math (per batch row b, C=1000 classes, T=4):
  x = y / T
  t3[i]   = (1/C) * sum_k w[k] * relu(x[i] - x[k])        # [C,C] intermediate, fused on-chip
  choice  = w * exp(-t3) + (1-w)
  ex      = exp(x)
  nsum    = sum_k (1-w[k]) * ex[k]
  p       = ex / (choice * nsum + ex)                      # p in (0, 1]
  (computed for y_s -> p_s and y_t -> p_t)
  pos     = |0.5*p_t - p_s + 0.5| ** 0.25
  neg     = |0.5*p_t - p_s|
  expt    = w ? pos : neg
  row[b]  = sum_i p_t * max(expt * ln(p_s), ln(1e-7))
  loss    = -mean_b(row) * T^2

Sharding: data-parallel over the batch dim. 64 rows -> 8 cores x 8 rows.
Each core returns its 8 row sums; the host averages and scales.

On-chip dataflow per core (8 rows, 2 inputs):
  - k padded 1000 -> 8 tiles x 128 (pad rows get w=0, contribute nothing)
  - x_bcast[128, 1000] built by a ones[1,128] matmul broadcast (PE), copied to SBUF
  - relu tile [128k, 1000i] = Relu(x_bcast + bias(-x_k)) on ACT (or DVE tensor_scalar)
  - PE contracts each relu tile with the w column (lhsT [128,1]) accumulating
    into PSUM acc[8, 1000] at partition offset b (two N=500 chains per row)
  - elementwise tail in [8, 1000] row layout, fused reductions via
    tensor_tensor_reduce / scalar_tensor_tensor accum_out
"""

import numpy as np

import concourse.bass as bass
import concourse.bacc as bacc
import concourse.tile as tile
from concourse import mybir
from concourse.bass_utils import run_bass_kernel_spmd
from concourse.masks import make_identity

B, C = 64, 1000
NCORES = 8
BPC = B // NCORES          # rows per core
P = 128                    # partitions
KT = 8                     # k tiles (7 full + 1 of 104)
KLAST = C - (KT - 1) * P   # 104
INV_T = 0.25               # 1/T
LOG_EPS = float(np.log(1e-7))

F32 = mybir.dt.float32
BF16 = mybir.dt.bfloat16
AF = mybir.ActivationFunctionType
ALU = mybir.AluOpType

# Producer engine per (input, batch, ktile) index: True -> DVE, False -> ACT.
# Flat index = (inp * BPC + b) * KT + kt.  Tuned after tracing.
N_PROD = 2 * BPC * KT


# producer split tuned from trace rates (ACT ~1.11us, DVE ~0.62us per tile;
# gpsimd tensor_scalar measured 23x slower -- never use it here)
def _producer_engine(flat_idx: int) -> str:
    return "act" if flat_idx % 8 in (0, 3, 6) else "dve"


def _ksz(kt: int) -> int:
    return KLAST if kt == KT - 1 else P


def build_kernel_body(ctx, tc, ys_d, yt_d, w_d, out_d):
    import os

    skip_mm = bool(int(os.environ.get("DISTILL_SKIP_MM", "0")))
    skip_relu = bool(int(os.environ.get("DISTILL_SKIP_RELU", "0")))
    skip_tp = bool(int(os.environ.get("DISTILL_SKIP_TP", "0")))
    nc = tc.nc
    consts = ctx.enter_context(tc.tile_pool(name="consts", bufs=1))
    rows = ctx.enter_context(tc.tile_pool(name="rows", bufs=1))
    cols = ctx.enter_context(tc.tile_pool(name="cols", bufs=1))
    xb_pool = ctx.enter_context(tc.tile_pool(name="xb", bufs=3))
    relu_pool = ctx.enter_context(tc.tile_pool(name="relu", bufs=8))
    tails = ctx.enter_context(tc.tile_pool(name="tails", bufs=1))
    ps_stage = ctx.enter_context(tc.tile_pool(name="ps_stage", bufs=2, space="PSUM"))
    ps_acc = ctx.enter_context(tc.tile_pool(name="ps_acc", bufs=1, space="PSUM"))

    # ---- constants ----
    ident = consts.tile([P, P], F32)
    make_identity(nc, ident)

    # ---- load rows, scale x = y/4 ----
    ys_raw = rows.tile([BPC, C], F32, tag="ys_raw", name="ys_raw")
    yt_raw = rows.tile([BPC, C], F32, tag="yt_raw", name="yt_raw")
    w_row = rows.tile([BPC, C], F32, tag="w_row", name="w_row")
    nc.sync.dma_start(out=ys_raw, in_=ys_d.ap())
    nc.sync.dma_start(out=yt_raw, in_=yt_d.ap())
    nc.sync.dma_start(out=w_row, in_=w_d.ap())

    x_rows = []
    for name, raw in (("xs", ys_raw), ("xt", yt_raw)):
        xr = rows.tile([BPC, C], F32, tag=name, name=name)
        nc.vector.tensor_scalar_mul(xr, raw, INV_T)
        x_rows.append(xr)

    # x rows bounced to DRAM: a 0-stride partition AP can broadcast a DRAM
    # row across all 128 partitions in one DMA (not legal from SBUF)
    dram = ctx.enter_context(tc.tile_pool(name="dram", bufs=1, space="DRAM"))
    xdr = dram.tile([2, BPC, C], F32, name="xdr")
    for inp in range(2):
        nc.sync.dma_start(out=xdr[inp], in_=x_rows[inp])

    omw = rows.tile([BPC, C], F32, tag="omw", name="omw")  # 1 - w
    nc.vector.tensor_scalar(
        omw, w_row, -1.0, 1.0, op0=ALU.mult, op1=ALU.add
    )

    # ---- column layouts via PE transpose: [128k, BPC] per ktile ----
    # negx_cols[inp][kt][:, b] = -x[b, kt*128 + p]
    # wplace[kt][:, b, m] = w[b, kt*128 + p] if m == b else 0   (stationary
    # [128, BPC] slices let the matmul write row b of the shared [BPC, C]
    # accumulator directly; other rows accumulate +0)
    negx_cols = [[], []]
    wplace = []
    for kt in range(KT):
        ksz = _ksz(kt)
        ksl = slice(kt * P, kt * P + ksz)
        for inp in range(2):
            pst = ps_stage.tile([P, BPC], F32, tag="xb_ps", name="pst")
            if skip_tp:
                nc.vector.memset(pst, 0.123)
            else:
                nc.tensor.transpose(pst[:ksz, :], x_rows[inp][:, ksl], ident[:BPC, :BPC])
            col = cols.tile([P, BPC], F32, tag=f"negx{inp}_{kt}", name=f"negx{inp}_{kt}")
            if ksz < P:
                # pad memset must start on a quadrant boundary; valid rows
                # 96:ksz are rewritten by the scalar_mul below
                nc.vector.memset(col[96:, :], 0.0)
            nc.vector.tensor_scalar_mul(col[:ksz, :], pst[:ksz, :], -1.0)
            negx_cols[inp].append(col)
        pst = ps_stage.tile([P, BPC], F32, tag="xb_ps", name="pst")
        if skip_tp:
            nc.vector.memset(pst, 0.5)
        else:
            nc.tensor.transpose(pst[:ksz, :], w_row[:, ksl], ident[:BPC, :BPC])
        wp = cols.tile([P, BPC, BPC], BF16, tag=f"wp_{kt}", name=f"wp_{kt}")
        nc.gpsimd.memset(wp, 0.0)
        # all 8 diagonal slots in one strided copy: flat free offset 9*b
        wp_flat = wp.rearrange("p a b -> p (a b)")
        diag = bass.AP(
            tensor=wp_flat.tensor,
            offset=wp_flat.offset,
            ap=[list(wp_flat.ap[0][:2]), [(BPC + 1) * wp_flat.ap[1][0], BPC]],
        )
        nc.vector.tensor_copy(diag[:ksz, :], pst[:ksz, :])
        wplace.append(wp)

    # ---- accumulators: acc[inp][b, i] = sum_k w[b,k] relu(x[b,i]-x[b,k]) ----
    accs = [
        ps_acc.tile([BPC, C], F32, tag="acc_s", name="acc_s"),
        ps_acc.tile([BPC, C], F32, tag="acc_t", name="acc_t"),
    ]

    if skip_mm:
        zacc = rows.tile([BPC, C], F32, tag="zacc", name="zacc")
        nc.vector.memset(zacc, 1.0)
        accs = [zacc, zacc]

    halves = ((0, 512), (512, 1000))  # psum bank = 512 f32; matmul can't cross
    for inp in range(2):
        for b in range(BPC):
            # broadcast x row b across all 128 partitions (0-stride DMA read)
            xb = xb_pool.tile([P, C], F32, tag="xb", name="xb")
            row = xdr[inp, b : b + 1, :]
            bc = bass.AP(
                tensor=row.tensor,
                offset=row.offset,
                ap=[[0, P]] + list(row.ap[1:]),
            )
            nc.sync.dma_start(out=xb, in_=bc)

            for kt in range(KT):
                rd = relu_pool.tile([P, C], BF16, tag="rd", name="rd")
                bias_col = negx_cols[inp][kt][:, b : b + 1]
                eng = _producer_engine((inp * BPC + b) * KT + kt)
                if skip_relu:
                    nc.vector.memset(rd, 0.25)
                elif eng == "dve":
                    nc.vector.tensor_scalar(
                        rd, xb, bias_col, 0.0, op0=ALU.add, op1=ALU.max
                    )
                elif eng == "gpsimd":
                    nc.gpsimd.tensor_scalar(
                        rd, xb, bias_col, 0.0, op0=ALU.add, op1=ALU.max
                    )
                else:
                    nc.scalar.activation(rd, xb, AF.Relu, bias=bias_col, scale=1.0)
                if not skip_mm:
                    for h0, h1 in halves:
                        nc.tensor.matmul(
                            accs[inp][:, h0:h1],
                            wplace[kt][:, b, :],
                            rd[:, h0:h1],
                            start=(b == 0 and kt == 0),
                            stop=(b == BPC - 1 and kt == KT - 1),
                        )

    # ---- elementwise tail in [BPC, C] layout ----
    def t(tag):
        return tails.tile([BPC, C], F32, tag=tag, name=tag)

    ps = []  # p_s, p_t
    exs = []  # exp(x_s), exp(x_t)
    for inp, nm in ((0, "s"), (1, "t")):
        ex = t(f"ex_{nm}")
        nc.scalar.activation(ex, x_rows[inp], AF.Exp)
        exs.append(ex)
        # nsum[b] = sum_k (1-w) * ex   (tensor_tensor_reduce wedges HW; use 2 ops)
        junk = t(f"junk_{nm}")
        nsum = tails.tile([BPC, 1], F32, tag=f"nsum_{nm}", name=f"nsum_{nm}")
        nc.gpsimd.tensor_mul(junk, ex, omw)
        nc.vector.reduce_sum(nsum, junk, axis=mybir.AxisListType.X)
        # choice = w * exp(-t3) + (1-w);  t3 = acc / C
        e3 = t(f"e3_{nm}")
        nc.scalar.activation(e3, accs[inp], AF.Exp, scale=-1.0 / C)
        ch = t(f"ch_{nm}")
        nc.gpsimd.tensor_mul(ch, e3, w_row)
        nc.gpsimd.tensor_add(ch, ch, omw)
        # den = choice * nsum + ex ; p = ex / den
        den = t(f"den_{nm}")
        nc.vector.scalar_tensor_tensor(
            out=den, in0=ch, scalar=nsum, in1=ex, op0=ALU.mult, op1=ALU.add
        )
        rden = t(f"rden_{nm}")
        rscr = t("rscr")
        nc.vector.reciprocal_approx_accurate(rden, den, rscr)
        p = t(f"p_{nm}")
        nc.gpsimd.tensor_mul(p, ex, rden)
        ps.append(p)

    p_s, p_t = ps
    # d1 = 0.5 * p_t - p_s
    d1 = t("d1")
    nc.vector.scalar_tensor_tensor(
        out=d1, in0=p_t, scalar=0.5, in1=p_s, op0=ALU.mult, op1=ALU.subtract
    )
    negv = t("negv")  # |d1| = max(-d1, d1)
    nc.vector.scalar_tensor_tensor(
        out=negv, in0=d1, scalar=-1.0, in1=d1, op0=ALU.mult, op1=ALU.max
    )
    posv = t("posv")  # |d1 + 0.5| ** 0.25
    half_bias = tails.tile([BPC, 1], F32, tag="half_bias", name="half_bias")
    nc.vector.memset(half_bias, 0.5)
    nc.scalar.activation(posv, d1, AF.Abs, bias=half_bias)
    nc.scalar.activation(posv, posv, AF.Sqrt)
    nc.scalar.activation(posv, posv, AF.Sqrt)
    # expt = w ? posv : negv  (w in {0,1}: negv + w*(posv-negv))
    expt = t("expt")
    nc.vector.tensor_sub(expt, posv, negv)
    nc.vector.tensor_mul(expt, expt, w_row)
    nc.vector.tensor_add(expt, expt, negv)
    # l = max(expt * ln(p_s), ln(1e-7)); row[b] = sum_i p_t * l
    lp = t("lp")
    nc.scalar.activation(lp, p_s, AF.Ln)
    l = t("l")
    nc.vector.tensor_mul(l, expt, lp)
    nc.vector.tensor_scalar_max(l, l, LOG_EPS)
    junk2 = t("junk2")
    rowsum = tails.tile([BPC, 1], F32, tag="rowsum", name="rowsum")
    nc.vector.tensor_mul(junk2, l, p_t)
    nc.vector.reduce_sum(rowsum, junk2, axis=mybir.AxisListType.X)
    nc.sync.dma_start(out=out_d.ap(), in_=rowsum)


_CACHE = {}


def _get_nc():
    if "nc" not in _CACHE:
        nc = bacc.Bacc(
            "TRN2", target_bir_lowering=False, debug=False, num_devices=NCORES
        )
        ys_d = nc.dram_tensor("y_s", [BPC, C], F32, kind="ExternalInput")
        yt_d = nc.dram_tensor("y_t", [BPC, C], F32, kind="ExternalInput")
        w_d = nc.dram_tensor("w", [BPC, C], F32, kind="ExternalInput")
        out_d = nc.dram_tensor("out", [BPC, 1], F32, kind="ExternalOutput")
        from contextlib import ExitStack

        with tile.TileContext(nc) as tc, ExitStack() as ctx:
            build_kernel_body(ctx, tc, ys_d, yt_d, w_d, out_d)
        nc.compile()
        _CACHE["nc"] = nc
    return _CACHE["nc"]


def kernel(y_s, y_t, w, _trace=False):
    nc = _get_nc()
    y_s = np.ascontiguousarray(y_s, dtype=np.float32)
    y_t = np.ascontiguousarray(y_t, dtype=np.float32)
    w = np.ascontiguousarray(w, dtype=np.float32)
    in_maps = [
        {
            "y_s": y_s[c * BPC : (c + 1) * BPC],
            "y_t": y_t[c * BPC : (c + 1) * BPC],
            "w": w[c * BPC : (c + 1) * BPC],
        }
        for c in range(NCORES)
    ]
    trace_kwargs = (
        {"trace": True, "trace_cores": list(range(NCORES))} if _trace else {}
    )
    res = run_bass_kernel_spmd(
        nc, in_maps, core_ids=list(range(NCORES)), **trace_kwargs
    )
    _CACHE["last_result"] = res
    rows = np.concatenate([res.results[c]["out"].reshape(-1) for c in range(NCORES)])
    return np.asarray(-16.0 * rows.mean(), dtype=np.float32)


# revision 27
# speedup vs baseline: 3.1194x; 1.1824x over previous
"""DistillKL ('w' branch) fused Trainium2 kernel.

Math (per batch row b, C=1000 classes, T=4):
  x = y / T
  t3[i]   = (1/C) * sum_k w[k] * relu(x[i] - x[k])        # [C,C] intermediate, fused on-chip
  choice  = w * exp(-t3) + (1-w)
  ex      = exp(x)
  nsum    = sum_k (1-w[k]) * ex[k]
  p       = ex / (choice * nsum + ex)                      # p in (0, 1]
  (computed for y_s -> p_s and y_t -> p_t)
  pos     = |0.5*p_t - p_s + 0.5| ** 0.25
  neg     = |0.5*p_t - p_s|
  expt    = w ? pos : neg
  row[b]  = sum_i p_t * max(expt * ln(p_s), ln(1e-7))
  loss    = -mean_b(row) * T^2

Sharding: data-parallel over the batch dim. 64 rows -> 8 cores x 8 rows.
Each core returns its 8 row sums; the host averages and scales.

On-chip dataflow per core (8 rows, 2 inputs):
  - k padded 1000 -> 8 tiles x 128 (pad rows get w=0, contribute nothing)
  - x_bcast[128, 1000] built by a ones[1,128] matmul broadcast (PE), copied to SBUF
  - relu tile [128k, 1000i] = Relu(x_bcast + bias(-x_k)) on ACT (or DVE tensor_scalar)
  - PE contracts each relu tile with the w column (lhsT [128,1]) accumulating
    into PSUM acc[8, 1000] at partition offset b (two N=500 chains per row)
  - elementwise tail in [8, 1000] row layout, fused reductions via
    tensor_tensor_reduce / scalar_tensor_tensor accum_out
"""

import numpy as np

import concourse.bass as bass
import concourse.bacc as bacc
import concourse.tile as tile
from concourse import mybir
from concourse.bass_utils import run_bass_kernel_spmd
from concourse.masks import make_identity

B, C = 64, 1000
NCORES = 8
BPC = B // NCORES          # rows per core
P = 128                    # partitions
KT = 8                     # k tiles (7 full + 1 of 104)
KLAST = C - (KT - 1) * P   # 104
INV_T = 0.25               # 1/T
LOG_EPS = float(np.log(1e-7))

F32 = mybir.dt.float32
BF16 = mybir.dt.bfloat16
AF = mybir.ActivationFunctionType
ALU = mybir.AluOpType

# Producer engine per (input, batch, ktile) index: True -> DVE, False -> ACT.
# Flat index = (inp * BPC + b) * KT + kt.  Tuned after tracing.
N_PROD = 2 * BPC * KT


# producer split tuned from trace rates (ACT ~1.11us, DVE ~0.62us per tile;
# gpsimd tensor_scalar measured 23x slower -- never use it here)
def _producer_engine(flat_idx: int) -> str:
    return "act" if flat_idx % 16 in (0, 2, 5, 8, 11, 13, 14) else "dve"


def _ksz(kt: int) -> int:
    return KLAST if kt == KT - 1 else P


def build_kernel_body(ctx, tc, ys_d, yt_d, w_d, out_d):
    import os

    skip_mm = bool(int(os.environ.get("DISTILL_SKIP_MM", "0")))
    skip_relu = bool(int(os.environ.get("DISTILL_SKIP_RELU", "0")))
    skip_tp = bool(int(os.environ.get("DISTILL_SKIP_TP", "0")))
    nc = tc.nc
    consts = ctx.enter_context(tc.tile_pool(name="consts", bufs=1))
    rows = ctx.enter_context(tc.tile_pool(name="rows", bufs=1))
    cols = ctx.enter_context(tc.tile_pool(name="cols", bufs=1))
    xb_pool = ctx.enter_context(tc.tile_pool(name="xb", bufs=4))
    relu_pool = ctx.enter_context(tc.tile_pool(name="relu", bufs=12))
    tails = ctx.enter_context(tc.tile_pool(name="tails", bufs=1))
    ps_stage = ctx.enter_context(tc.tile_pool(name="ps_stage", bufs=2, space="PSUM"))
    ps_acc = ctx.enter_context(tc.tile_pool(name="ps_acc", bufs=1, space="PSUM"))

    # ---- constants ----
    ident = consts.tile([P, P], F32)
    make_identity(nc, ident)

    # ---- load rows, scale x = y/4 ----
    ys_raw = rows.tile([BPC, C], F32, tag="ys_raw", name="ys_raw")
    yt_raw = rows.tile([BPC, C], F32, tag="yt_raw", name="yt_raw")
    w_row = rows.tile([BPC, C], F32, tag="w_row", name="w_row")
    nc.sync.dma_start(out=ys_raw, in_=ys_d.ap())
    nc.sync.dma_start(out=yt_raw, in_=yt_d.ap())
    nc.sync.dma_start(out=w_row, in_=w_d.ap())

    x_rows = []
    for name, raw in (("xs", ys_raw), ("xt", yt_raw)):
        xr = rows.tile([BPC, C], F32, tag=name, name=name)
        nc.vector.tensor_scalar_mul(xr, raw, INV_T)
        x_rows.append(xr)

    # x rows bounced to DRAM: a 0-stride partition AP can broadcast a DRAM
    # row across all 128 partitions in one DMA (not legal from SBUF)
    dram = ctx.enter_context(tc.tile_pool(name="dram", bufs=1, space="DRAM"))
    xdr = dram.tile([2, BPC, C], F32, name="xdr")
    for inp in range(2):
        nc.sync.dma_start(out=xdr[inp], in_=x_rows[inp])

    omw = rows.tile([BPC, C], F32, tag="omw", name="omw")  # 1 - w
    nc.vector.tensor_scalar(
        omw, w_row, -1.0, 1.0, op0=ALU.mult, op1=ALU.add
    )

    # ---- column layouts via PE transpose: [128k, BPC] per ktile ----
    # negx_cols[inp][kt][:, b] = -x[b, kt*128 + p]
    # wplace[kt][:, b, m] = w[b, kt*128 + p] if m == b else 0   (stationary
    # [128, BPC] slices let the matmul write row b of the shared [BPC, C]
    # accumulator directly; other rows accumulate +0)
    negx_cols = [[], []]
    wplace = []
    for kt in range(KT):
        ksz = _ksz(kt)
        ksl = slice(kt * P, kt * P + ksz)
        for inp in range(2):
            pst = ps_stage.tile([P, BPC], F32, tag="xb_ps", name="pst")
            if skip_tp:
                nc.vector.memset(pst, 0.123)
            else:
                nc.tensor.transpose(pst[:ksz, :], x_rows[inp][:, ksl], ident[:BPC, :BPC])
            col = cols.tile([P, BPC], F32, tag=f"negx{inp}_{kt}", name=f"negx{inp}_{kt}")
            if ksz < P:
                # pad memset must start on a quadrant boundary; valid rows
                # 96:ksz are rewritten by the scalar_mul below
                nc.vector.memset(col[96:, :], 0.0)
            nc.vector.tensor_scalar_mul(col[:ksz, :], pst[:ksz, :], -1.0)
            negx_cols[inp].append(col)
        pst = ps_stage.tile([P, BPC], F32, tag="xb_ps", name="pst")
        if skip_tp:
            nc.vector.memset(pst, 0.5)
        else:
            nc.tensor.transpose(pst[:ksz, :], w_row[:, ksl], ident[:BPC, :BPC])
        wp = cols.tile([P, BPC, BPC], BF16, tag=f"wp_{kt}", name=f"wp_{kt}")
        nc.gpsimd.memset(wp, 0.0)
        # all 8 diagonal slots in one strided copy: flat free offset 9*b
        wp_flat = wp.rearrange("p a b -> p (a b)")
        diag = bass.AP(
            tensor=wp_flat.tensor,
            offset=wp_flat.offset,
            ap=[list(wp_flat.ap[0][:2]), [(BPC + 1) * wp_flat.ap[1][0], BPC]],
        )
        nc.vector.tensor_copy(diag[:ksz, :], pst[:ksz, :])
        wplace.append(wp)

    # ---- accumulators: acc[inp][b, i] = sum_k w[b,k] relu(x[b,i]-x[b,k]) ----
    accs = [
        ps_acc.tile([BPC, C], F32, tag="acc_s", name="acc_s"),
        ps_acc.tile([BPC, C], F32, tag="acc_t", name="acc_t"),
    ]

    if skip_mm:
        zacc = rows.tile([BPC, C], F32, tag="zacc", name="zacc")
        nc.vector.memset(zacc, 1.0)
        accs = [zacc, zacc]

    halves = ((0, 512), (512, 1000))  # psum bank = 512 f32; matmul can't cross
    for inp in range(2):
        for b in range(BPC):
            # broadcast x row b across all 128 partitions (0-stride DMA read)
            xb = xb_pool.tile([P, C], F32, tag="xb", name="xb")
            row = xdr[inp, b : b + 1, :]
            bc = bass.AP(
                tensor=row.tensor,
                offset=row.offset,
                ap=[[0, P]] + list(row.ap[1:]),
            )
            nc.sync.dma_start(out=xb, in_=bc)

            for kt in range(KT):
                rd = relu_pool.tile([P, C], BF16, tag="rd", name="rd")
                bias_col = negx_cols[inp][kt][:, b : b + 1]
                eng = _producer_engine((inp * BPC + b) * KT + kt)
                if skip_relu:
                    nc.vector.memset(rd, 0.25)
                elif eng == "dve":
                    nc.vector.tensor_scalar(
                        rd, xb, bias_col, 0.0, op0=ALU.add, op1=ALU.max
                    )
                elif eng == "gpsimd":
                    nc.gpsimd.tensor_scalar(
                        rd, xb, bias_col, 0.0, op0=ALU.add, op1=ALU.max
                    )
                else:
                    nc.scalar.activation(rd, xb, AF.Relu, bias=bias_col, scale=1.0)
                if not skip_mm:
                    for h0, h1 in halves:
                        nc.tensor.matmul(
                            accs[inp][:, h0:h1],
                            wplace[kt][:, b, :],
                            rd[:, h0:h1],
                            start=(b == 0 and kt == 0),
                            stop=(b == BPC - 1 and kt == KT - 1),
                        )

    # ---- elementwise tail in [BPC, C] layout ----
    def t(tag):
        return tails.tile([BPC, C], F32, tag=tag, name=tag)

    ps = []  # p_s, p_t
    exs = []  # exp(x_s), exp(x_t)
    for inp, nm in ((0, "s"), (1, "t")):
        ex = t(f"ex_{nm}")
        nc.scalar.activation(ex, x_rows[inp], AF.Exp)
        exs.append(ex)
        # nsum[b] = sum_k (1-w) * ex   (tensor_tensor_reduce wedges HW; use 2 ops)
        junk = t(f"junk_{nm}")
        nsum = tails.tile([BPC, 1], F32, tag=f"nsum_{nm}", name=f"nsum_{nm}")
        nc.vector.scalar_tensor_tensor(
            out=junk, in0=ex, scalar=1.0, in1=omw, op0=ALU.bypass, op1=ALU.mult,
            accum_out=nsum,
        )
        # choice = w * exp(-t3) + (1-w);  t3 = acc / C
        e3 = t(f"e3_{nm}")
        nc.scalar.activation(e3, accs[inp], AF.Exp, scale=-1.0 / C)
        ch = t(f"ch_{nm}")
        nc.vector.tensor_mul(ch, e3, w_row)
        nc.vector.tensor_add(ch, ch, omw)
        # den = choice * nsum + ex ; p = ex / den
        den = t(f"den_{nm}")
        nc.vector.scalar_tensor_tensor(
            out=den, in0=ch, scalar=nsum, in1=ex, op0=ALU.mult, op1=ALU.add
        )
        rden = t(f"rden_{nm}")
        nc.vector.reciprocal_approx_fast(rden, den)
        p = t(f"p_{nm}")
        nc.vector.tensor_mul(p, ex, rden)
        ps.append(p)

    p_s, p_t = ps
    # d1 = 0.5 * p_t - p_s
    d1 = t("d1")
    nc.vector.scalar_tensor_tensor(
        out=d1, in0=p_t, scalar=0.5, in1=p_s, op0=ALU.mult, op1=ALU.subtract
    )
    negv = t("negv")  # |d1| = max(-d1, d1)
    nc.vector.scalar_tensor_tensor(
        out=negv, in0=d1, scalar=-1.0, in1=d1, op0=ALU.mult, op1=ALU.max
    )
    posv = t("posv")  # |d1 + 0.5| ** 0.25
    half_bias = tails.tile([BPC, 1], F32, tag="half_bias", name="half_bias")
    nc.vector.memset(half_bias, 0.5)
    nc.scalar.activation(posv, d1, AF.Abs, bias=half_bias)
    nc.scalar.activation(posv, posv, AF.Sqrt)
    nc.scalar.activation(posv, posv, AF.Sqrt)
    # expt = w ? posv : negv  (w in {0,1}: negv + w*(posv-negv))
    expt = t("expt")
    nc.vector.tensor_sub(expt, posv, negv)
    nc.vector.tensor_mul(expt, expt, w_row)
    nc.vector.tensor_add(expt, expt, negv)
    # l = max(expt * ln(p_s), ln(1e-7)); row[b] = sum_i p_t * l
    lp = t("lp")
    nc.scalar.activation(lp, p_s, AF.Ln)
    l = t("l")
    nc.vector.tensor_mul(l, expt, lp)
    # fused: rowsum = sum(max(l, LOG_EPS) * p_t)
    junk2 = t("junk2")
    rowsum = tails.tile([BPC, 1], F32, tag="rowsum", name="rowsum")
    nc.vector.scalar_tensor_tensor(
        out=junk2, in0=l, scalar=LOG_EPS, in1=p_t, op0=ALU.max, op1=ALU.mult,
        accum_out=rowsum,
    )
    nc.sync.dma_start(out=out_d.ap(), in_=rowsum)


_CACHE = {}


def _get_nc():
    if "nc" not in _CACHE:
        nc = bacc.Bacc(
            "TRN2", target_bir_lowering=False, debug=False, num_devices=NCORES
        )
        ys_d = nc.dram_tensor("y_s", [BPC, C], F32, kind="ExternalInput")
        yt_d = nc.dram_tensor("y_t", [BPC, C], F32, kind="ExternalInput")
        w_d = nc.dram_tensor("w", [BPC, C], F32, kind="ExternalInput")
        out_d = nc.dram_tensor("out", [BPC, 1], F32, kind="ExternalOutput")
        from contextlib import ExitStack

        with tile.TileContext(nc) as tc, ExitStack() as ctx:
            build_kernel_body(ctx, tc, ys_d, yt_d, w_d, out_d)
        nc.compile()
        _CACHE["nc"] = nc
    return _CACHE["nc"]


def kernel(y_s, y_t, w, _trace=False):
    nc = _get_nc()
    y_s = np.ascontiguousarray(y_s, dtype=np.float32)
    y_t = np.ascontiguousarray(y_t, dtype=np.float32)
    w = np.ascontiguousarray(w, dtype=np.float32)
    in_maps = [
        {
            "y_s": y_s[c * BPC : (c + 1) * BPC],
            "y_t": y_t[c * BPC : (c + 1) * BPC],
            "w": w[c * BPC : (c + 1) * BPC],
        }
        for c in range(NCORES)
    ]
    trace_kwargs = (
        {"trace": True, "trace_cores": list(range(NCORES))} if _trace else {}
    )
    res = run_bass_kernel_spmd(
        nc, in_maps, core_ids=list(range(NCORES)), **trace_kwargs
    )
    _CACHE["last_result"] = res
    rows = np.concatenate([res.results[c]["out"].reshape(-1) for c in range(NCORES)])
    return np.asarray(-16.0 * rows.mean(), dtype=np.float32)
